# revision 1
# baseline (speedup 1.0000x reference)
"""Trainium2 Bass kernel for a 3-layer relu-LSTM classifier.

Architecture (per core, data-parallel over batch across 8 cores, B=16 each):
  x = emb[tokens]                       (indirect-DMA gather, 128 tokens/block)
  xg_l = x_l @ W_l + b_l  (bulk, PE)    -> DRAM, "folded transposed" layout
  recurrence per step (For_i):  g.T = U_l.T @ h.T  (PE, bf16 weights)
      gates/state kept as [128 partitions, nk*16] folded layout so the
      DVE/ACT elementwise ops use all 128 partitions.
  dense head on-device, output sigmoid [16] f32 per core.

Self-contained: hardcodes all shapes; host side only reformats weights
(permutation/fold/bf16 cast) and shards tokens.
"""

import os

import numpy as np
import ml_dtypes

BF16 = ml_dtypes.bfloat16

# Model dims
NCORES = 8
B_TOT, T = 128, 512
B = B_TOT // NCORES  # 16
VOCAB, EMB_D = 5000, 300
EMB_PAD = 384  # padded to 3*128
UNITS = [256, 512, 256]
DENSE = 64

# Per-layer derived dims
# layer l: u, d_in(padded), nk = u//128 (contraction tiles / h fold slabs),
# nm = 4*nk (gate m-tiles), F = nk*16 (fold width), FW = 4*F (g fold width)
LCFG = []
_d = EMB_PAD
for _u in UNITS:
    _nk = _u // 128
    LCFG.append(dict(u=_u, d=_d, nkw=_d // 128, nk=_nk, nm=4 * _nk,
                     F=_nk * 16, FW=4 * _nk * 16))
    _d = _u

TC = 32          # time-steps per bulk-projection chunk (N = TC*16 = 512)
NCHUNK = T // TC
UNROLL = 4       # rec loop half-steps per For_i body (must be even)
STAGGERED = os.environ.get("K_STAGGERED", "1") == "1"

_CACHE = {}
LAST_RESULT = None  # BassKernelResults of the most recent run (for test.py)


def gate_perm(u):
    """Column permutation of [i f cc o]-ordered 4u gate dim into our
    m-tile order: blocks (f, cc, i, o), each block j-minor over u//128."""
    nk = u // 128
    base = [1, 0, 3, 2]  # block order (f, i, o, cc); keras gate idx (i=0, f=1, cc=2, o=3)
    perm = np.empty(4 * u, dtype=np.int64)
    for blk in range(4):
        for j in range(nk):
            m = blk * nk + j
            perm[m * 128:(m + 1) * 128] = base[blk] * u + j * 128 + np.arange(128)
    return perm


def fold_lhs(Wp, nkt, nm):
    """[nkt*128, nm*128] -> [128, nkt*nm*128] with tile (k, m) at cols
    ((k*nm)+m)*128."""
    K, M = Wp.shape
    assert K == nkt * 128 and M == nm * 128, (Wp.shape, nkt, nm)
    return np.ascontiguousarray(
        Wp.reshape(nkt, 128, nm, 128).transpose(1, 0, 2, 3).reshape(128, nkt * nm * 128)
    )


def prep_weights(inputs):
    """Host-side reformatting of the model weights (shared by all cores)."""
    f32 = lambda x: np.asarray(x, dtype=np.float32)
    out = {}
    perms = [gate_perm(u) for u in UNITS]
    W0 = np.zeros((EMB_PAD, 4 * UNITS[0]), np.float32)
    W0[:EMB_D] = f32(inputs["W0"])
    Ws = [W0, f32(inputs["W1"]), f32(inputs["W2"])]
    for l in range(3):
        cfg = LCFG[l]
        p = perms[l]
        out[f"w{l}"] = fold_lhs(Ws[l][:, p], cfg["nkw"], cfg["nm"]).astype(BF16)
        out[f"u{l}"] = fold_lhs(f32(inputs[f"U{l}"])[:, p], cfg["nk"], cfg["nm"]).astype(BF16)
        out[f"b{l}"] = np.ascontiguousarray(
            f32(inputs[f"b{l}"])[p].reshape(cfg["nm"], 128).T)
    Wd = f32(inputs["Wd"])  # [256, 64]
    out["wd"] = np.concatenate([Wd[0:128], Wd[128:256]], axis=1).astype(BF16)  # [128,128]
    out["bd"] = f32(inputs["bd"])           # [64]
    out["wc"] = f32(inputs["Wc"]).astype(BF16)  # [64, 1]
    out["bc"] = f32(inputs["bc"])           # [1]
    return out


def build_program():
    from concourse import bacc
    import concourse.mybir as mybir
    import concourse.tile as tile
    from concourse.bass import ds

    FP32 = mybir.dt.float32
    BF = mybir.dt.bfloat16
    AF = mybir.ActivationFunctionType
    ALU = mybir.AluOpType

    nc = bacc.Bacc(None, target_bir_lowering=False)

    # ---- DRAM parameters ------------------------------------------------
    tok_d = nc.declare_dram_parameter("tokens_tb", [T * B], mybir.dt.int32, isOutput=False)
    emb_d = nc.declare_dram_parameter("emb", [VOCAB, EMB_D], FP32, isOutput=False)
    wp = {}
    for l in range(3):
        cfg = LCFG[l]
        wp[f"w{l}"] = nc.declare_dram_parameter(f"w{l}", [128, cfg["nkw"] * cfg["nm"] * 128], BF, isOutput=False)
        wp[f"u{l}"] = nc.declare_dram_parameter(f"u{l}", [128, cfg["nk"] * cfg["nm"] * 128], BF, isOutput=False)
        wp[f"b{l}"] = nc.declare_dram_parameter(f"b{l}", [128, cfg["nm"]], FP32, isOutput=False)
    wd_d = nc.declare_dram_parameter("wd", [128, 128], BF, isOutput=False)
    bd_d = nc.declare_dram_parameter("bd", [DENSE], FP32, isOutput=False)
    wc_d = nc.declare_dram_parameter("wc", [DENSE, 1], BF, isOutput=False)
    bc_d = nc.declare_dram_parameter("bc", [1], FP32, isOutput=False)
    out_d = nc.declare_dram_parameter("out", [B], FP32, isOutput=True)

    # ---- internal DRAM scratch (padded for recurrence xg prefetch) ------
    xg_d = [nc.dram_tensor(f"xg{l}", [128, (T + 4 * UNROLL) * LCFG[l]["FW"]], BF)
            for l in range(3)]

    from concourse.masks import make_identity

    with tile.TileContext(nc) as tc:
        stk = []

        def pool(name, bufs, space="SBUF"):
            return tc.tile_pool(name=name, bufs=bufs, space=space)

        with pool("const", 1) as constp:
            ident = constp.tile([128, 128], FP32)
            make_identity(nc, ident[:])
            identb = constp.tile([128, 128], BF)
            make_identity(nc, identb[:])
            tok_sb = constp.tile([128, (T * B) // 128], mybir.dt.int32)
            nc.sync.dma_start(tok_sb[:], tok_d[:].rearrange("(i p) -> p i", p=128))
            bias_sb = []
            for l in range(3):
                bt = constp.tile([128, LCFG[l]["nm"]], FP32, tag=f"bias{l}")
                nc.sync.dma_start(bt[:], wp[f"b{l}"][:])
                bias_sb.append(bt)
            wd_sb = constp.tile([128, 128], BF)
            nc.sync.dma_start(wd_sb[:], wd_d[:])
            bd_sb = constp.tile([DENSE, 1], FP32)
            nc.sync.dma_start(bd_sb[:], bd_d[:])
            wc_sb = constp.tile([DENSE, 1], BF)
            nc.sync.dma_start(wc_sb[:], wc_d[:])
            bc_sb = constp.tile([1, 1], FP32)
            nc.sync.dma_start(bc_sb[:], bc_d[:])

            # ============ Phase A: gather + transpose -> xT =============
            NTOK = T * B           # 8192
            NBLK = NTOK // 128     # 64
            with pool("xT", 1) as xtp:
                xT = xtp.tile([128, 3 * NTOK], BF)
                # zero slab k=2 (rows 44: stay zero; 0:44 overwritten below)
                nc.gpsimd.memset(xT[:, 2 * NTOK:3 * NTOK], 0.0)
                with nc.named_scope("gather_transpose"):
                    with pool("gath", 3) as gp, pool("tps", 2, "PSUM") as tpp:
                        for blk in range(NBLK):
                            xb = gp.tile([128, EMB_PAD], FP32, tag="xb")
                            import concourse.bass as bass_mod
                            nc.gpsimd.indirect_dma_start(
                                out=xb[:, 0:EMB_D], out_offset=None,
                                in_=emb_d[:, :],
                                in_offset=bass_mod.IndirectOffsetOnAxis(
                                    ap=tok_sb[:, blk:blk + 1], axis=0),
                            )
                            for k in range(3):
                                tps = tpp.tile([128, 128], FP32, tag="tps")
                                nc.tensor.transpose(tps[:], xb[:, 128 * k:128 * (k + 1)], ident[:])
                                rows = 128 if k < 2 else 44
                                nc.vector.tensor_copy(
                                    out=xT[0:rows, k * NTOK + 128 * blk: k * NTOK + 128 * (blk + 1)],
                                    in_=tps[0:rows, :])

                # ============ Phase B: xg0 bulk =============
                _bulk_proj(nc, tc, pool, 0, wp["w0"], bias_sb[0], xg_d[0],
                           rhs_fn=lambda k, c: xT[:, k * NTOK + c * 512: k * NTOK + (c + 1) * 512])

            # ============ Phase C: L0 recurrence =============
            with pool("seq0", 1) as sq0:
                h0_seq = sq0.tile([128, (T + 1) * LCFG[0]["F"]], BF)
                _recurrence(nc, tc, pool, 0, wp["u0"], xg_d[0], h0_seq, ds, identb)

                # ============ Phase D: xg1 bulk =============
                F0 = LCFG[0]["F"]
                h0r = h0_seq[:].rearrange("p (s w) -> p s w", w=F0)
                _bulk_proj(nc, tc, pool, 1, wp["w1"], bias_sb[1], xg_d[1],
                           rhs_fn=lambda k, c: h0r[:, c * TC + 1: (c + 1) * TC + 1, k * 16:(k + 1) * 16])

            # ============ Phase E: L1 recurrence =============
            with pool("seq1", 1) as sq1:
                h1_seq = sq1.tile([128, (T + 1) * LCFG[1]["F"]], BF)
                _recurrence(nc, tc, pool, 1, wp["u1"], xg_d[1], h1_seq, ds, identb)

                # ============ Phase F: xg2 bulk =============
                F1 = LCFG[1]["F"]
                h1r = h1_seq[:].rearrange("p (s w) -> p s w", w=F1)
                _bulk_proj(nc, tc, pool, 2, wp["w2"], bias_sb[2], xg_d[2],
                           rhs_fn=lambda k, c: h1r[:, c * TC + 1: (c + 1) * TC + 1, k * 16:(k + 1) * 16])

            # ============ Phase G: L2 recurrence =============
            hb2 = _recurrence(nc, tc, pool, 2, wp["u2"], xg_d[2], None, ds, identb)

            # ============ Phase H: dense head =============
            with nc.named_scope("dense"):
                F2 = LCFG[2]["F"]
                with pool("dps", 1, "PSUM") as dpp:
                    psd = dpp.tile([DENSE, 16], FP32, tag="psd")
                    for k in range(2):
                        nc.tensor.matmul(psd[:], lhsT=wd_sb[:, 64 * k:64 * (k + 1)],
                                         rhs=hb2[:, F2 + 16 * k:F2 + 16 * (k + 1)],
                                         start=(k == 0), stop=(k == 1))
                    hd = constp.tile([DENSE, 16], BF, tag="hd")
                    nc.scalar.activation(hd[:], psd[:], AF.Relu, bias=bd_sb[:, 0:1])
                    psc = dpp.tile([1, 16], FP32, tag="psc")
                    nc.tensor.matmul(psc[:], lhsT=wc_sb[:], rhs=hd[:], start=True, stop=True)
                    outv = constp.tile([1, 16], FP32, tag="outv")
                    nc.scalar.activation(outv[:], psc[:], AF.Sigmoid, bias=bc_sb[0:1, 0:1])
                    nc.sync.dma_start(out_d[:], outv[0:1, :])

    nc.finalize()
    return nc


def _bulk_proj(nc, tc, pool, l, w_dram, bias_sb, xg_dram, rhs_fn):
    """xg_l[:, t*FW + m*16 + b] over a chunked [TC*16]-token loop.
    rhs_fn(k, chunk) -> [128, 512]-sized AP of the (transposed) layer input."""
    import concourse.mybir as mybir
    FP32 = mybir.dt.float32
    BF = mybir.dt.bfloat16
    ALU = mybir.AluOpType
    cfg = LCFG[l]
    nkw, nm, FW = cfg["nkw"], cfg["nm"], cfg["FW"]
    with nc.named_scope(f"xg{l}_bulk"):
        with pool(f"w{l}p", 1) as wpool, pool(f"xps{l}", 2, "PSUM") as xpp, \
                pool(f"stage{l}", 2) as stp:
            w_sb = wpool.tile([128, nkw * nm * 128], BF)
            nc.sync.dma_start(w_sb[:], w_dram[:])
            for c in range(NCHUNK):
                stage = stp.tile([128, TC * FW], BF, tag="stage")
                stager = stage[:].rearrange("p (t w) -> p t w", w=FW)
                for m in range(nm):
                    ps = xpp.tile([128, 512], FP32, tag="xps")
                    for k in range(nkw):
                        nc.tensor.matmul(
                            ps[:], lhsT=w_sb[:, ((k * nm) + m) * 128:((k * nm) + m + 1) * 128],
                            rhs=rhs_fn(k, c), start=(k == 0), stop=(k == nkw - 1))
                    nc.vector.tensor_scalar(
                        out=stager[:, :, m * 16:(m + 1) * 16],
                        in0=ps[:].rearrange("p (t b) -> p t b", b=16),
                        scalar1=bias_sb[:, m:m + 1], scalar2=None, op0=ALU.add)
                nc.sync.dma_start(xg_dram[:, c * TC * FW:(c + 1) * TC * FW], stage[:])


def _recurrence(nc, tc, pool, l, u_dram, xg_dram, h_seq, ds, identb):
    """Run the T-step LSTM recurrence for layer l. Returns the ping/pong h
    tile (final h in slab p=1). Writes h into h_seq slots 1..T if given.

    Body covers 2*UNROLL steps with two xg prefetch buffers (A/B) so the
    xg slab DMA is always a body ahead. The per-step xg add is injected
    into the PSUM accumulation via an identity-stationary matmul; sigmoid
    reads PSUM directly."""
    import concourse.mybir as mybir
    FP32 = mybir.dt.float32
    BF = mybir.dt.bfloat16
    AF = mybir.ActivationFunctionType
    ALU = mybir.AluOpType
    cfg = LCFG[l]
    nk, nm, F, FW = cfg["nk"], cfg["nm"], cfg["F"], cfg["FW"]
    HALF = 4 if l == 1 else 8  # steps per prefetch half-body
    U2 = 2 * HALF              # steps per body

    with nc.named_scope(f"rec{l}"):
        with pool(f"u{l}p", 1) as upool, pool(f"st{l}", 1) as statep:
            u_sb = upool.tile([128, nk * nm * 128], BF)
            nc.sync.dma_start(u_sb[:], u_dram[:])
            hb = statep.tile([128, 2 * F], BF, tag="hb")
            cbuf = statep.tile([128, F], FP32, tag="cb")
            warm = statep.tile([1, 1], FP32, tag="warm")
            xga = statep.tile([128, HALF * FW], BF, tag="xga")
            xgb = statep.tile([128, HALF * FW], BF, tag="xgb")
            nc.gpsimd.memset(hb[:], 0.0)
            nc.gpsimd.memset(cbuf[:], 0.0)
            if h_seq is not None:
                nc.gpsimd.memset(h_seq[:, 0:F], 0.0)
            # touch the sigmoid table before the loop so the per-iteration
            # ACT_TABLE_LOAD hoists out of the loop body
            nc.scalar.activation(warm[:], cbuf[0:1, 0:1], AF.Sigmoid)
            # preload first half-body's xg slabs
            nc.sync.dma_start(xga[:], xg_dram[:, 0:HALF * FW])

            tc.strict_bb_all_engine_barrier()

            def step(po, xgbuf):
                p = po % 2
                sl = po % HALF
                ps_fio = ppa.tile([128, 3 * F], FP32, tag="psfio")
                ps_cc = ppc.tile([128, F], FP32, tag="pscc")
                xg_sl = xgbuf[:, sl * FW:sl * FW + FW]
                # inject xg into PSUM, then accumulate U k-tiles
                nc.tensor.matmul(ps_fio[:], lhsT=identb[:], rhs=xg_sl[:, 0:3 * F],
                                 start=True, stop=False, skip_group_check=True)
                for m in range(3 * nk):
                    dst = ps_fio[:, m * 16:(m + 1) * 16]
                    for k in range(nk):
                        nc.tensor.matmul(
                            dst, lhsT=u_sb[:, ((k * nm) + m) * 128:((k * nm) + m + 1) * 128],
                            rhs=hb[:, (1 - p) * F + k * 16:(1 - p) * F + (k + 1) * 16],
                            start=False, stop=(k == nk - 1), skip_group_check=True)
                nc.tensor.matmul(ps_cc[:], lhsT=identb[:], rhs=xg_sl[:, 3 * F:FW],
                                 start=True, stop=False, skip_group_check=True)
                for m in range(3 * nk, nm):
                    dst = ps_cc[:, (m - 3 * nk) * 16:(m - 3 * nk + 1) * 16]
                    for k in range(nk):
                        nc.tensor.matmul(
                            dst, lhsT=u_sb[:, ((k * nm) + m) * 128:((k * nm) + m + 1) * 128],
                            rhs=hb[:, (1 - p) * F + k * 16:(1 - p) * F + (k + 1) * 16],
                            start=False, stop=(k == nk - 1), skip_group_check=True)
                # gates: f=[0:F], i=[F:2F], o=[2F:3F] in ps_fio; cc in ps_cc
                sfio = tmp.tile([128, 3 * F], FP32, tag="sfio")
                nc.scalar.activation(sfio[:, 0:2 * F], ps_fio[:, 0:2 * F], AF.Sigmoid)
                c2 = tmp.tile([128, F], FP32, tag="c2")
                nc.gpsimd.tensor_mul(out=c2[:], in0=cbuf[:], in1=sfio[:, 0:F])
                t1 = tmp.tile([128, F], FP32, tag="t1")
                nc.vector.scalar_tensor_tensor(
                    out=t1[:], in0=ps_cc[:], scalar=0.0, in1=sfio[:, F:2 * F],
                    op0=ALU.max, op1=ALU.mult)
                nc.scalar.activation(sfio[:, 2 * F:3 * F], ps_fio[:, 2 * F:3 * F], AF.Sigmoid)
                nc.vector.tensor_add(out=cbuf[:], in0=c2[:], in1=t1[:])
                nc.vector.scalar_tensor_tensor(
                    out=hb[:, p * F:(p + 1) * F], in0=cbuf[:], scalar=0.0,
                    in1=sfio[:, 2 * F:3 * F], op0=ALU.max, op1=ALU.mult)

            def hflush(i, po):
                # after odd steps: slots (i+po, i+po+1) = hb[0:2F]
                if h_seq is not None:
                    nc.sync.dma_start(h_seq[:, ds((i + po) * F, 2 * F)], hb[:])

            with pool(f"rp{l}a", 2, "PSUM") as ppa, pool(f"rp{l}c", 2, "PSUM") as ppc, \
                    pool(f"rt{l}", 2) as tmp:
                hint = (mybir.EngineType.PE, mybir.EngineType.SP)
                with tc.For_i(0, T, U2, staggered_reset=STAGGERED,
                              hint_engines=hint) as i:
                    # A holds [i, i+U); prefetch [i+U, i+2U) into B, compute
                    # from A; reload A <- [i+2U, i+3U) for the next body.
                    nc.sync.dma_start(xgb[:], xg_dram[:, ds((i + HALF) * FW, HALF * FW)])
                    for po in range(HALF):
                        step(po, xga)
                        if po % 2 == 1:
                            hflush(i, po)
                    nc.sync.dma_start(xga[:], xg_dram[:, ds((i + 2 * HALF) * FW, HALF * FW)])
                    for po in range(HALF, U2):
                        step(po, xgb)
                        if po % 2 == 1:
                            hflush(i, po)
            return hb


def _get_program():
    if "nc" not in _CACHE:
        _CACHE["nc"] = build_program()
    return _CACHE["nc"]


def kernel(**inputs):
    global LAST_RESULT
    from concourse.bass_utils import run_bass_kernel_spmd

    nc = _get_program()
    w = prep_weights(inputs)
    tokens = np.asarray(inputs["tokens"], dtype=np.int32)  # [128, 512]

    in_maps = []
    for core in range(NCORES):
        tk = tokens[core * B:(core + 1) * B]          # [16, 512]
        tok_tb = np.ascontiguousarray(tk.T).reshape(-1)  # t-major: idx = t*16+b
        m = {"tokens_tb": tok_tb,
             "emb": np.asarray(inputs["emb"], dtype=np.float32)}
        m.update(w)
        in_maps.append(m)

    trace = os.environ.get("K_TRACE", "0") == "1"
    res = run_bass_kernel_spmd(nc, in_maps, list(range(NCORES)), trace=trace)
    LAST_RESULT = res
    out = np.concatenate([res.results[c]["out"].reshape(B, 1) for c in range(NCORES)], axis=0)
    return out.astype(np.float32)



# revision 8
# speedup vs baseline: 2.2328x; 2.2328x over previous
"""Trainium2 Bass kernel for a 3-layer relu-LSTM classifier.

Data-parallel over batch across 8 cores (B=16/core).  Single fused
software-pipelined wavefront: per 16-step chunk c the kernel runs

  SG : DMA-copy xT token slab for chunk c+2 into a ring (static APs)
  S0 : xg0 = xT @ W0 + b0 for chunk c+1           (PE bulk, N=256)
  S1 : L0 recurrence steps of chunk c
  S2 : xg1 = h0 @ W1 + b1 for chunk c-1           (PE bulk)
  S3 : L1 recurrence steps of chunk c-2
  S4 : xg2 = h1 @ W2 + b2 for chunk c-3           (PE bulk)
  S5 : L2 recurrence steps of chunk c-4

interleaved at per-timestep granularity, so each layer's elementwise
chain (1 ACT sigmoid + 4 DVE ops) hides under the other layers' matmul
streams.  All xg/h traffic stays in SBUF rings; nothing bounces through
DRAM.  Gates live in one PSUM tile [128, 4F] per step, order (f,i,o,cc),
injected via identity matmul.

Self-contained: hardcodes all shapes; host side only reformats weights.
"""

import os

import numpy as np
import ml_dtypes

BF16 = ml_dtypes.bfloat16

# Model dims
NCORES = 8
B_TOT, T = 128, 512
B = B_TOT // NCORES  # 16
VOCAB, EMB_D = 5000, 300
EMB_PAD = 384  # padded to 3*128
UNITS = [256, 512, 256]
DENSE = 64

# Wavefront geometry
TCr = 16             # timesteps per chunk
NCH = T // TCr       # 32 chunks
NTOK = T * B         # 8192 tokens per core
NBLK = NTOK // 128   # 64 gather blocks
CW = TCr * B         # 256 token columns per chunk

# Per-layer derived dims
LCFG = []
_d = EMB_PAD
for _u in UNITS:
    _nk = _u // 128
    LCFG.append(dict(u=_u, d=_d, nkw=_d // 128, nk=_nk, nm=4 * _nk,
                     F=_nk * 16, FW=4 * _nk * 16))
    _d = _u

STAGGERED = os.environ.get("K_STAGGERED", "1") == "1"

_CACHE = {}
LAST_RESULT = None  # BassKernelResults of the most recent run (for test.py)


def gate_perm(u):
    """Column permutation of [i f cc o]-ordered 4u gate dim into our
    m-tile order: blocks (f, i, o, cc), each block j-minor over u//128."""
    nk = u // 128
    base = [1, 0, 3, 2]  # keras gate idx (i=0, f=1, cc=2, o=3) -> (f, i, o, cc)
    perm = np.empty(4 * u, dtype=np.int64)
    for blk in range(4):
        for j in range(nk):
            m = blk * nk + j
            perm[m * 128:(m + 1) * 128] = base[blk] * u + j * 128 + np.arange(128)
    return perm


def fold_lhs(Wp, nkt, nm):
    """[nkt*128, nm*128] -> [128, nkt*nm*128] with tile (k, m) at cols
    ((k*nm)+m)*128."""
    K, M = Wp.shape
    assert K == nkt * 128 and M == nm * 128, (Wp.shape, nkt, nm)
    return np.ascontiguousarray(
        Wp.reshape(nkt, 128, nm, 128).transpose(1, 0, 2, 3).reshape(128, nkt * nm * 128)
    )


def prep_weights(inputs):
    """Host-side reformatting of the model weights (shared by all cores)."""
    f32 = lambda x: np.asarray(x, dtype=np.float32)
    out = {}
    perms = [gate_perm(u) for u in UNITS]
    W0 = np.zeros((EMB_PAD, 4 * UNITS[0]), np.float32)
    W0[:EMB_D] = f32(inputs["W0"])
    Ws = [W0, f32(inputs["W1"]), f32(inputs["W2"])]
    for l in range(3):
        cfg = LCFG[l]
        p = perms[l]
        out[f"w{l}"] = fold_lhs(Ws[l][:, p], cfg["nkw"], cfg["nm"]).astype(BF16)
        out[f"u{l}"] = fold_lhs(f32(inputs[f"U{l}"])[:, p], cfg["nk"], cfg["nm"]).astype(BF16)
        out[f"b{l}"] = np.ascontiguousarray(
            f32(inputs[f"b{l}"])[p].reshape(cfg["nm"], 128).T)
    Wd = f32(inputs["Wd"])  # [256, 64]
    out["wd"] = np.concatenate([Wd[0:128], Wd[128:256]], axis=1).astype(BF16)  # [128,128]
    out["bd"] = f32(inputs["bd"])           # [64]
    out["wc"] = f32(inputs["Wc"]).astype(BF16)  # [64, 1]
    out["bc"] = f32(inputs["bc"])           # [1]
    return out


def build_program():
    from concourse import bacc
    import concourse.mybir as mybir
    import concourse.tile as tile
    import concourse.bass as bass_mod
    from concourse.bass import ds
    from concourse.masks import make_identity

    FP32 = mybir.dt.float32
    BF = mybir.dt.bfloat16
    AF = mybir.ActivationFunctionType
    ALU = mybir.AluOpType

    nc = bacc.Bacc(None, target_bir_lowering=False)

    # ---- DRAM parameters ------------------------------------------------
    tok_d = nc.declare_dram_parameter("tokens_tb", [T * B], mybir.dt.int32, isOutput=False)
    emb_d = nc.declare_dram_parameter("emb", [VOCAB, EMB_D], FP32, isOutput=False)
    wp = {}
    for l in range(3):
        cfg = LCFG[l]
        wp[f"w{l}"] = nc.declare_dram_parameter(f"w{l}", [128, cfg["nkw"] * cfg["nm"] * 128], BF, isOutput=False)
        wp[f"u{l}"] = nc.declare_dram_parameter(f"u{l}", [128, cfg["nk"] * cfg["nm"] * 128], BF, isOutput=False)
        wp[f"b{l}"] = nc.declare_dram_parameter(f"b{l}", [128, cfg["nm"]], FP32, isOutput=False)
    wd_d = nc.declare_dram_parameter("wd", [128, 128], BF, isOutput=False)
    bd_d = nc.declare_dram_parameter("bd", [DENSE], FP32, isOutput=False)
    wc_d = nc.declare_dram_parameter("wc", [DENSE, 1], BF, isOutput=False)
    bc_d = nc.declare_dram_parameter("bc", [1], FP32, isOutput=False)
    out_d = nc.declare_dram_parameter("out", [B], FP32, isOutput=True)

    F0, F1, F2 = LCFG[0]["F"], LCFG[1]["F"], LCFG[2]["F"]
    FW0, FW1, FW2 = LCFG[0]["FW"], LCFG[1]["FW"], LCFG[2]["FW"]

    with tile.TileContext(nc) as tc:
        def pool(name, bufs, space="SBUF"):
            return tc.tile_pool(name=name, bufs=bufs, space=space)

        with pool("const", 1) as constp, pool("wts", 1) as wtp, \
                pool("xT", 1) as xtp:
            ident = constp.tile([128, 128], FP32)
            make_identity(nc, ident[:])
            identb = constp.tile([128, 128], BF)
            make_identity(nc, identb[:])
            tok_sb = constp.tile([128, NBLK], mybir.dt.int32)
            nc.sync.dma_start(tok_sb[:], tok_d[:].rearrange("(i p) -> p i", p=128))
            bias_sb = []
            for l in range(3):
                bt = constp.tile([128, LCFG[l]["nm"]], FP32, tag=f"bias{l}")
                nc.sync.dma_start(bt[:], wp[f"b{l}"][:])
                bias_sb.append(bt)
            wd_sb = constp.tile([128, 128], BF)
            nc.sync.dma_start(wd_sb[:], wd_d[:])
            bd_sb = constp.tile([DENSE, 1], FP32)
            nc.sync.dma_start(bd_sb[:], bd_d[:])
            wc_sb = constp.tile([DENSE, 1], BF)
            nc.sync.dma_start(wc_sb[:], wc_d[:])
            bc_sb = constp.tile([1, 1], FP32)
            nc.sync.dma_start(bc_sb[:], bc_d[:])

            # weight tiles (resident)
            w_sb = {}
            for l in range(3):
                cfg = LCFG[l]
                wt = wtp.tile([128, cfg["nkw"] * cfg["nm"] * 128], BF, tag=f"w{l}")
                nc.sync.dma_start(wt[:], wp[f"w{l}"][:])
                w_sb[f"w{l}"] = wt
                ut = wtp.tile([128, cfg["nk"] * cfg["nm"] * 128], BF, tag=f"u{l}")
                nc.sync.dma_start(ut[:], wp[f"u{l}"][:])
                w_sb[f"u{l}"] = ut

            # ============ Phase A: gather + transpose -> xT =============
            # xT layout: slab k in cols [k*NTOK, (k+1)*NTOK), token-major.
            # Padded by 2*CW cols for harmless OOB prefetch at the tail.
            xT = xtp.tile([128, 3 * NTOK + 2 * CW], BF)
            nc.gpsimd.memset(xT[:, 2 * NTOK:3 * NTOK + 2 * CW], 0.0)
            with nc.named_scope("gatherA"):
                with pool("gath", 3) as gp, pool("tps", 2, "PSUM") as tpp:
                    for blk in range(NBLK):
                        xb = gp.tile([128, EMB_PAD], FP32, tag="xb")
                        nc.gpsimd.indirect_dma_start(
                            out=xb[:, 0:EMB_D], out_offset=None,
                            in_=emb_d[:, :],
                            in_offset=bass_mod.IndirectOffsetOnAxis(
                                ap=tok_sb[:, blk:blk + 1], axis=0),
                        )
                        for k in range(3):
                            tps = tpp.tile([128, 128], FP32, tag="tps")
                            nc.tensor.transpose(tps[:], xb[:, 128 * k:128 * (k + 1)], ident[:])
                            rows = 128 if k < 2 else 44
                            nc.vector.tensor_copy(
                                out=xT[0:rows, k * NTOK + 128 * blk: k * NTOK + 128 * (blk + 1)],
                                in_=tps[0:rows, :])

            # ============ Wavefront =============
            with pool("rings", 1) as rp, pool("state", 1) as stp, \
                    pool("tmp0", 2) as tp0, pool("tmp1", 2) as tp1, pool("tmp2", 2) as tp2:

                xTr = rp.tile([128, 2 * 3 * CW], BF, tag="xTr")       # SG ring
                x0r = rp.tile([128, 2 * TCr * FW0], BF, tag="x0r")   # xg0 ring
                x1r = rp.tile([128, 2 * TCr * FW1], BF, tag="x1r")   # xg1 ring
                x2r = rp.tile([128, 2 * TCr * FW2], BF, tag="x2r")   # xg2 ring
                h0r = rp.tile([128, 2 * TCr * F0], BF, tag="h0r")    # h0 ring
                h1r = rp.tile([128, 2 * TCr * F1], BF, tag="h1r")    # h1 ring
                h2b = stp.tile([128, 2 * F2], BF, tag="h2b")         # h2 ping-pong
                cb0 = stp.tile([128, LCFG[0]["F"]], FP32, tag="cb0")
                cb1 = stp.tile([128, LCFG[1]["F"]], FP32, tag="cb1")
                cb2 = stp.tile([128, LCFG[2]["F"]], FP32, tag="cb2")
                cb = [cb0, cb1, cb2]
                warm = stp.tile([1, 1], FP32, tag="warm")

                # zero initial h/c state
                nc.gpsimd.memset(h0r[:, (TCr + TCr - 1) * F0:(2 * TCr) * F0], 0.0)
                nc.gpsimd.memset(h1r[:, (TCr + TCr - 1) * F1:(2 * TCr) * F1], 0.0)
                nc.gpsimd.memset(h2b[:, F2:2 * F2], 0.0)
                for l in range(3):
                    nc.gpsimd.memset(cb[l][:], 0.0)
                # hoist the sigmoid ACT table load out of the loop
                nc.scalar.activation(warm[:], cb[0][0:1, 0:1], AF.Sigmoid)

                _ps_cms = [pool("ps0", 2, "PSUM"), pool("ps1", 2, "PSUM"),
                           pool("ps2", 2, "PSUM"), pool("bps", 2, "PSUM")]
                pp0, pp1, pp2, bpp = [p.__enter__() for p in _ps_cms]

                tmpp = [tp0, tp1, tp2]
                psp = [pp0, pp1, pp2]
                hrings = [h0r, h1r, None]
                xgrings = [x0r, x1r, x2r]

                def rec_step(l, c, s):
                    """One LSTM step of layer l at (chunk c, step s)."""
                    cfg = LCFG[l]
                    F, FW, nk, nm = cfg["F"], cfg["FW"], cfg["nk"], cfg["nm"]
                    u_sb = w_sb[f"u{l}"]
                    slot = c % 2
                    xg = xgrings[l]
                    xg_sl = xg[:, (slot * TCr + s) * FW:(slot * TCr + s + 1) * FW]
                    if l < 2:
                        hr = hrings[l]
                        if s == 0:
                            h_prev = hr[:, (((c - 1) % 2) * TCr + TCr - 1) * F:
                                         (((c - 1) % 2) * TCr + TCr) * F]
                        else:
                            h_prev = hr[:, (slot * TCr + s - 1) * F:(slot * TCr + s) * F]
                        h_out = hr[:, (slot * TCr + s) * F:(slot * TCr + s + 1) * F]
                    else:
                        h_prev = h2b[:, ((s - 1) % 2) * F2:((s - 1) % 2 + 1) * F2]
                        h_out = h2b[:, (s % 2) * F2:(s % 2 + 1) * F2]

                    ps = psp[l].tile([128, FW], FP32, tag=f"ps{l}")
                    nc.tensor.matmul(ps[:], lhsT=identb[:], rhs=xg_sl,
                                     start=True, stop=False, skip_group_check=True)
                    for m in range(nm):
                        dst = ps[:, m * 16:(m + 1) * 16]
                        for k in range(nk):
                            nc.tensor.matmul(
                                dst, lhsT=u_sb[:, ((k * nm) + m) * 128:((k * nm) + m + 1) * 128],
                                rhs=h_prev[:, k * 16:(k + 1) * 16],
                                start=False, stop=(k == nk - 1), skip_group_check=True)
                    sfio = tmpp[l].tile([128, 3 * F], FP32, tag=f"sf{l}")
                    nc.scalar.activation(sfio[:], ps[:, 0:3 * F], AF.Sigmoid)
                    t1 = tmpp[l].tile([128, F], FP32, tag=f"t1{l}")
                    nc.vector.scalar_tensor_tensor(
                        out=t1[:], in0=ps[:, 3 * F:4 * F], scalar=0.0,
                        in1=sfio[:, F:2 * F], op0=ALU.max, op1=ALU.mult)
                    c2 = tmpp[l].tile([128, F], FP32, tag=f"c2{l}")
                    nc.vector.tensor_mul(out=c2[:], in0=cb[l][:], in1=sfio[:, 0:F])
                    nc.vector.tensor_add(out=cb[l][:], in0=c2[:], in1=t1[:])
                    nc.vector.scalar_tensor_tensor(
                        out=h_out, in0=cb[l][:], scalar=0.0,
                        in1=sfio[:, 2 * F:3 * F], op0=ALU.max, op1=ALU.mult)

                def proj_slice(pl, m, csrc):
                    """One m-tile of the xg{pl} chunk-projection for chunk csrc."""
                    cfg = LCFG[pl]
                    nkw, nm, FW = cfg["nkw"], cfg["nm"], cfg["FW"]
                    wt = w_sb[f"w{pl}"]
                    slot = csrc % 2
                    ps = bpp.tile([128, CW], FP32, tag="bps")
                    for k in range(nkw):
                        if pl == 0:
                            rhs = xTr[:, slot * 3 * CW + k * CW: slot * 3 * CW + (k + 1) * CW]
                        else:
                            Fs = LCFG[pl - 1]["F"]
                            hsrc = hrings[pl - 1][:].rearrange("p (s w) -> p s w", w=Fs)
                            rhs = hsrc[:, slot * TCr:(slot + 1) * TCr, k * 16:(k + 1) * 16]
                        nc.tensor.matmul(ps[:], lhsT=wt[:, ((k * nm) + m) * 128:((k * nm) + m + 1) * 128],
                                         rhs=rhs, start=(k == 0), stop=(k == nkw - 1))
                    dst = xgrings[pl][:].rearrange("p (s w) -> p s w", w=FW)
                    nc.vector.tensor_scalar(
                        out=dst[:, slot * TCr:(slot + 1) * TCr, m * 16:(m + 1) * 16],
                        in0=ps[:].rearrange("p (s b) -> p s b", b=16),
                        scalar1=bias_sb[pl][:, m:m + 1], scalar2=None, op0=ALU.add)

                def sg_copy(cdst, ioff):
                    """DMA xT token slab of chunk cdst into xTr ring (3 slabs)."""
                    slot = cdst % 2
                    for k in range(3):
                        if ioff is None:
                            src = xT[:, k * NTOK + cdst * CW: k * NTOK + (cdst + 1) * CW]
                        else:
                            src = xT[:, ds(ioff * CW + (k * NTOK), CW)]
                        nc.sync.dma_start(
                            xTr[:, slot * 3 * CW + k * CW: slot * 3 * CW + (k + 1) * CW], src)

                def emit_master(c, ioff=None):
                    """One master-chunk position of the wavefront.

                    c: python int for ring-slot math (and static APs when
                    ioff is None); ioff: For_i index expression for the SG
                    DMA source offset (equals c symbolically)."""
                    sg = c + 2 <= (33 if ioff is not None else 31)
                    s0 = c + 1 <= (32 if ioff is not None else 31)
                    s1 = c <= 31
                    s2 = 0 <= c - 1 <= 31
                    s3 = 0 <= c - 2 <= 31
                    s4 = 0 <= c - 3 <= 31
                    s5 = 0 <= c - 4 <= 31
                    for s in range(TCr):
                        if s == 0 and sg:
                            sg_copy(c + 2, None if ioff is None else ioff + 2)
                        if s1:
                            rec_step(0, c, s)
                        if s0 and s % 2 == 0:
                            proj_slice(0, s // 2, c + 1)
                        if s2:
                            proj_slice(1, s, c - 1)
                        if s3:
                            rec_step(1, c - 2, s)
                        if s4 and s % 2 == 0:
                            proj_slice(2, s // 2, c - 3)
                        if s5:
                            rec_step(2, c - 4, s)

                with nc.named_scope("wave"):
                    # pipeline fill
                    sg_copy(0, None)
                    sg_copy(1, None)
                    for s in range(0, TCr, 2):
                        proj_slice(0, s // 2, 0)
                    for c in range(4):
                        emit_master(c)
                    # steady state: uniform chunks 4..31, two per iteration
                    hint = (mybir.EngineType.PE, mybir.EngineType.SP)
                    with tc.For_i(4, 32, 2, staggered_reset=STAGGERED,
                                  hint_engines=hint) as i:
                        emit_master(4, ioff=i)
                        emit_master(5, ioff=i + 1)
                    # drain
                    for c in range(32, 36):
                        emit_master(c)

                for p in reversed(_ps_cms):
                    p.__exit__(None, None, None)

                # ============ dense head =============
                with nc.named_scope("dense"):
                    with pool("dps", 1, "PSUM") as dpp:
                        psd = dpp.tile([DENSE, 16], FP32, tag="psd")
                        for k in range(2):
                            nc.tensor.matmul(psd[:], lhsT=wd_sb[:, 64 * k:64 * (k + 1)],
                                             rhs=h2b[:, F2 + 16 * k:F2 + 16 * (k + 1)],
                                             start=(k == 0), stop=(k == 1))
                        hd = constp.tile([DENSE, 16], BF, tag="hd")
                        nc.scalar.activation(hd[:], psd[:], AF.Relu, bias=bd_sb[:, 0:1])
                        psc = dpp.tile([1, 16], FP32, tag="psc")
                        nc.tensor.matmul(psc[:], lhsT=wc_sb[:], rhs=hd[:], start=True, stop=True)
                        outv = constp.tile([1, 16], FP32, tag="outv")
                        nc.scalar.activation(outv[:], psc[:], AF.Sigmoid, bias=bc_sb[0:1, 0:1])
                        nc.sync.dma_start(out_d[:], outv[0:1, :])

    nc.finalize()
    return nc


def _get_program():
    if "nc" not in _CACHE:
        _CACHE["nc"] = build_program()
    return _CACHE["nc"]


def kernel(**inputs):
    global LAST_RESULT
    from concourse.bass_utils import run_bass_kernel_spmd

    nc = _get_program()
    w = prep_weights(inputs)
    tokens = np.asarray(inputs["tokens"], dtype=np.int32)  # [128, 512]

    in_maps = []
    for core in range(NCORES):
        tk = tokens[core * B:(core + 1) * B]          # [16, 512]
        tok_tb = np.ascontiguousarray(tk.T).reshape(-1)  # t-major: idx = t*16+b
        m = {"tokens_tb": tok_tb,
             "emb": np.asarray(inputs["emb"], dtype=np.float32)}
        m.update(w)
        in_maps.append(m)

    trace = os.environ.get("K_TRACE", "0") == "1"
    res = run_bass_kernel_spmd(nc, in_maps, list(range(NCORES)), trace=trace)
    LAST_RESULT = res
    out = np.concatenate([res.results[c]["out"].reshape(B, 1) for c in range(NCORES)], axis=0)
    return out.astype(np.float32)
